# revision 40
# baseline (speedup 1.0000x reference)
"""Decoder block (rmsnorm->MHA(rope on Q,V)->W_O residual->rmsnorm->MLP residual)
on 8 Trainium2 NeuronCores.

Sharding: each core computes attention for 2 of the 16 heads over BOTH batches
(weights sharded by head), then AllToAll redistributes head outputs so each
core owns one (batch, 512-token-block) slice for the W_O projection, second
rmsnorm and MLP (full weights, token-sharded). Host concatenates the 8
token-block outputs (transposing each [D,TB] slab).

v4 structure (1258us -> ~1005us on HW):
- attention is HEAD-MAJOR: h=0 chains for both batches run first so the h=0
  AllToAll fires at ~50% of attention and hides under the h=1 chains; the
  h=1 AllToAll tail is covered by W_O pass A (even-head chunks, weights
  preloaded during attention; odd-head pass B after).
- score chunks processed in pairs: two score matmuls land in adjacent PSUM
  banks of one [P,2,TB] tile and ONE exp activation covers both, halving the
  scalar-engine overhead that paces the long causal chains.
- softmax denominator: DVE pairwise running sum of the exp tiles + one
  ones_sq-matmul per chain whose output lands PRE-BROADCAST on all 128
  partitions, so the reciprocal is a fast full-width approx op and gpsimd
  stays off the critical path. The whole tail is deferred into the next
  chain so the in-order PE/DVE queues never block on it.
- phases 3/4 fully transposed: W_O emits x2^T directly (lhsT=W_O tile,
  rhs=head-outputs), rmsnorm2 reduces over partitions via a ones_sq-matmul,
  MLP2 emits out^T; no PE transposes, no x2 DRAM roundtrip.
- the post-collective staging DMAs ride the scalar queue, gated by a token
  copy off the last attention tile so the scheduler cannot float their
  semaphore wait ahead of remaining attention work.
- xT / W1 / W2 are host-relayouted into per-block slabs so every DMA line is
  16KB (full HBM efficiency) instead of 1KB/256B strided reads.
- startup: x-row DMAs + stats first, weights behind them; warmup matmuls are
  N=512 and span the initial DMA window so HAM stays warm.
"""

import os

import numpy as np

B, S, D, H = 2, 2048, 2048, 16
DH = 128
NC = 8
HPC = 2  # heads per core
P = 128
TB = 512  # token block (= S/4) and q-chunk width
KC = D // P  # 16 contraction chunks over D
FC = (4 * D) // P  # 64 contraction chunks over the MLP hidden dim
EPS = 1e-8
THETA = 10000.0

_CACHE = {}


def _install_ntff_hook():
    """Optional: register the axon NTFF profiling hook so trace=True works."""
    import sys
    import types

    if "antenv.axon_hooks" in sys.modules:
        return True
    try:
        mod = types.ModuleType("antenv.axon_hooks")
        _hook = [None]
        mod.set_axon_ntff_profile_hook = lambda h: _hook.__setitem__(0, h)
        mod.get_axon_ntff_profile_hook = lambda: _hook[0]
        import antenv
        from trn_agent_boot.trn_boot import _ntff_profile_via_ctypes

        sys.modules["antenv.axon_hooks"] = mod
        antenv.axon_hooks = mod
        mod.set_axon_ntff_profile_hook(
            _ntff_profile_via_ctypes("/opt/axon/libaxon_pjrt.so")
        )
        return True
    except Exception:
        return False


def _build():
    import concourse.bass as bass
    import concourse.mybir as mybir
    import concourse.tile as tile
    from concourse import bacc
    from concourse.masks import make_identity
    from contextlib import ExitStack

    f32 = mybir.dt.float32
    f16 = mybir.dt.float16
    AF = mybir.ActivationFunctionType
    OP = mybir.AluOpType

    nc = bacc.Bacc("TRN2", target_bir_lowering=False, debug=False, num_devices=NC)

    xT3_d = nc.dram_tensor("xT3", [B * 4, P, KC, TB], f16, kind="ExternalInput")
    xf_d = nc.dram_tensor("xf", [B * S, D], f16, kind="ExternalInput")
    xrT_d = nc.dram_tensor("xrT", [D, TB], f16, kind="ExternalInput")
    wq = nc.dram_tensor("wq", [D, HPC * P], f16, kind="ExternalInput")
    wk = nc.dram_tensor("wk", [D, HPC * P], f16, kind="ExternalInput")
    wv = nc.dram_tensor("wv", [D, HPC * P], f16, kind="ExternalInput")
    wo = nc.dram_tensor("wo", [D, D], f16, kind="ExternalInput")
    w1r = nc.dram_tensor("w1r", [16, P, KC, TB], f16, kind="ExternalInput")
    w2r = nc.dram_tensor("w2r", [KC, P, FC, P], f16, kind="ExternalInput")
    b1s = nc.dram_tensor("b1s", [P, FC], f32, kind="ExternalInput")
    b2s = nc.dram_tensor("b2s", [P, KC], f32, kind="ExternalInput")
    cos_qt = nc.dram_tensor("cos_qt", [64, S], f16, kind="ExternalInput")
    sin_qt = nc.dram_tensor("sin_qt", [64, S], f16, kind="ExternalInput")
    cos_v = nc.dram_tensor("cos_v", [S, 64], f16, kind="ExternalInput")
    sin_v = nc.dram_tensor("sin_v", [S, 64], f16, kind="ExternalInput")
    masks = nc.dram_tensor("masks", [4, P, TB], f16, kind="ExternalInput")
    out_d = nc.dram_tensor("out", [D, TB], f32, kind="ExternalOutput")

    inv_sqrt_dh = float(1.0 / np.sqrt(DH))

    with tile.TileContext(nc) as tc, ExitStack() as ctx:
        cst = ctx.enter_context(tc.tile_pool(name="cst", bufs=1))
        dram = ctx.enter_context(tc.tile_pool(name="dram", bufs=1, space="DRAM"))
        # long-lived across phases 3-4
        h2Tp = ctx.enter_context(tc.tile_pool(name="h2Tp", bufs=1))

        eps_t = cst.tile([P, 1], f32)
        nc.vector.memset(eps_t, EPS)
        ident16 = cst.tile([P, P], f16)
        make_identity(nc, ident16)
        ones_sq = cst.tile([P, P], f16)
        nc.vector.memset(ones_sq, 1.0)
        warm_rhs = cst.tile([P, TB], f16)
        nc.vector.memset(warm_rhs, 0.0)
        b1_sb = cst.tile([P, FC], f32)
        # pass-A phase-3 inputs, hoisted so their DMAs can run during
        # attention (no SBUF region reuse -> no false WAR dependencies)
        woeAt = cst.tile([P, 8, D], f16)
        hoTe = cst.tile([P, 8, TB], f16)
        with tc.tile_pool(name="wrm", bufs=1, space="PSUM") as wrmp:
            wrm = wrmp.tile([P, TB], f32)
            for _ in range(85):
                nc.tensor.matmul(wrm, ident16, warm_rhs, start=True, stop=True)

        # internal DRAM for the collectives
        a2a_in0 = dram.tile([NC, P, TB], f16, name="a2a_in0")
        a2a_out0 = dram.tile([NC, P, TB], f16, name="a2a_out0")
        a2a_in1 = dram.tile([NC, P, TB], f16, name="a2a_in1")
        a2a_out1 = dram.tile([NC, P, TB], f16, name="a2a_out1")

        # ---------- phase 1+2: rmsnorm1 fused with QKV/attention ----------
        with ExitStack() as p2:
            # spanning pools: Q/K/V for BOTH batches (head-major attention
            # order needs them concurrently) + the attention mask
            qrk = p2.enter_context(tc.tile_pool(name="qrk", bufs=1))
            vsb = p2.enter_context(tc.tile_pool(name="vsb", bufs=1))
            mskp = p2.enter_context(tc.tile_pool(name="mskp", bufs=1))
            maskt = mskp.tile([P, 4, TB], f16)

            pj = ExitStack()
            xTp = pj.enter_context(tc.tile_pool(name="xTp", bufs=2))
            xfp = pj.enter_context(tc.tile_pool(name="xfp", bufs=2))
            smp = pj.enter_context(tc.tile_pool(name="smp", bufs=4))
            rsqp = pj.enter_context(tc.tile_pool(name="rsqp", bufs=1))
            diagp = pj.enter_context(tc.tile_pool(name="diagp", bufs=2))
            cqsp = pj.enter_context(tc.tile_pool(name="cqsp", bufs=2))
            vcsp = pj.enter_context(tc.tile_pool(name="vcsp", bufs=2))
            acst = pj.enter_context(tc.tile_pool(name="acst", bufs=1))
            rtmp = pj.enter_context(tc.tile_pool(name="rtmp", bufs=1))
            vtmp = pj.enter_context(tc.tile_pool(name="vtmp", bufs=1))
            qkps = pj.enter_context(tc.tile_pool(name="qkps", bufs=2, space="PSUM"))
            vps = pj.enter_context(tc.tile_pool(name="vps", bufs=1, space="PSUM"))

            # tile declarations for the deferred weight/table DMAs (emitted
            # after the first stats DMAs so the startup path drains first)
            wq_sb = acst.tile([P, KC, HPC * P], f16)
            wk_sb = acst.tile([P, KC, HPC * P], f16)
            wv_sb = acst.tile([P, KC, HPC * P], f16)
            cosq = acst.tile([64, S], f16)
            sinq = acst.tile([64, S], f16)
            cosv = acst.tile([P, KC, 64], f16)
            sinv = acst.tile([P, KC, 64], f16)

            rsqa = {}
            for b in range(B):
                rsqa[b] = rsqp.tile([P, KC], f32, tag=f"rsq{b}", name=f"rsq{b}")
            diags = {}
            QR = {}
            KK = {}
            VV = {}

            def emit_stats(b, qc):
                # rms stats for the 4 token-chunks of block (b, qc); Squares
                # batched before Sqrt/recip so the activation table loads once
                diag = diagp.tile([P, 4, P], f16, tag=f"dg{b}", name=f"dg{b}_{qc}")
                ssqB = smp.tile([P, 4], f32, tag="ssq", name=f"ssq{b}_{qc}")
                for i in range(4):
                    g = qc * 4 + i
                    xfr = xfp.tile([P, D], f16, tag="xf", name=f"xf{b}_{g}")
                    nc.scalar.dma_start(
                        xfr, xf_d.ap()[b * S + g * P : b * S + (g + 1) * P, :]
                    )
                    nc.scalar.activation(
                        xfr, xfr, AF.Square, accum_out=ssqB[:, i : i + 1]
                    )
                rmsB = smp.tile([P, 4], f32, tag="rms", name=f"rms{b}_{qc}")
                nc.scalar.activation(
                    rmsB, ssqB, AF.Sqrt, bias=eps_t, scale=float(1.0 / D)
                )
                nc.vector.reciprocal(rsqa[b][:, qc * 4 : (qc + 1) * 4], rmsB)
                for i in range(4):
                    g = qc * 4 + i
                    nc.vector.tensor_scalar_mul(
                        diag[:, i, :], ident16, rsqa[b][:, g : g + 1]
                    )
                diags[(b, qc)] = diag

            def load_xT(b, qc):
                xTt = xTp.tile([P, KC, TB], f16, tag="xT", name=f"xT{b}_{qc}")
                nc.sync.dma_start(xTt, xT3_d.ap()[b * 4 + qc])
                return xTt

            def emit_proj(b, qc, xTt=None):
                if xTt is None:
                    xTt = load_xT(b, qc)
                # rsqB[p, q] = rsq per token q, on all partitions p
                rsqB = qkps.tile([P, TB], f32, tag="qk", name=f"rB{b}_{qc}")
                nc.tensor.matmul(
                    rsqB,
                    ones_sq,
                    diags.pop((b, qc)).rearrange("p a b -> p (a b)"),
                    start=True,
                    stop=True,
                )
                qslc = slice(qc * TB, (qc + 1) * TB)
                rsqBs = cqsp.tile([P, TB], f16, tag="rBs", name=f"rBs{b}_{qc}")
                nc.vector.tensor_copy(rsqBs, rsqB)
                cqs = cqsp.tile([64, TB], f16, tag="cqs", name=f"cqs{b}_{qc}")
                sqs = cqsp.tile([64, TB], f16, tag="sqs", name=f"sqs{b}_{qc}")
                nc.vector.tensor_mul(cqs, cosq[:, qslc], rsqBs[0:64, :])
                nc.vector.tensor_mul(sqs, sinq[:, qslc], rsqBs[0:64, :])

                for h in range(HPC):
                    # Q projection + rope (even dims 0:64 = x1, odd = x2);
                    # rmsnorm scale folded into cqs/sqs
                    qp = qkps.tile([P, TB], f32, tag="qk", name=f"qp{b}{qc}{h}")
                    for d in range(KC):
                        nc.tensor.matmul(
                            qp,
                            wq_sb[:, d, h * P : (h + 1) * P],
                            xTt[:, d, :],
                            start=(d == 0),
                            stop=(d == KC - 1),
                        )
                    t1 = rtmp.tile([64, TB], f16, tag="t1", name=f"t1_{b}{qc}{h}")
                    t2 = rtmp.tile([64, TB], f16, tag="t2", name=f"t2_{b}{qc}{h}")
                    t3 = rtmp.tile([64, TB], f16, tag="t3", name=f"t3_{b}{qc}{h}")
                    t4 = rtmp.tile([64, TB], f16, tag="t4", name=f"t4_{b}{qc}{h}")
                    nc.vector.tensor_mul(t1, qp[0:64, :], cqs)
                    nc.vector.tensor_mul(t2, qp[64:P, :], sqs)
                    nc.vector.tensor_tensor(QR[b, h][0:64, qslc], t1, t2, OP.subtract)
                    nc.vector.tensor_mul(t3, qp[0:64, :], sqs)
                    nc.vector.tensor_mul(t4, qp[64:P, :], cqs)
                    nc.vector.tensor_tensor(QR[b, h][64:P, qslc], t3, t4, OP.add)
                    # K projection: raw copy; rmsnorm scale rides exp()
                    kp = qkps.tile([P, TB], f32, tag="qk", name=f"kp{b}{qc}{h}")
                    for d in range(KC):
                        nc.tensor.matmul(
                            kp,
                            wk_sb[:, d, h * P : (h + 1) * P],
                            xTt[:, d, :],
                            start=(d == 0),
                            stop=(d == KC - 1),
                        )
                    # rmsnorm scale of the K side applied here (per token col)
                    nc.vector.tensor_mul(KK[b, h][:, qslc], kp, rsqBs)
                # V projection + rope, natural layout [tok, head, dh];
                # rmsnorm scale folded into the per-chunk rope tables
                for tt in range(4):
                    gt_ = qc * 4 + tt
                    vp_ = vps.tile([P, HPC, P], f32, tag="v", name=f"vp{b}_{qc}_{tt}")
                    for d in range(KC):
                        nc.tensor.matmul(
                            vp_.rearrange("p h k -> p (h k)"),
                            xTt[:, d, tt * P : (tt + 1) * P],
                            wv_sb[:, d, :],
                            start=(d == 0),
                            stop=(d == KC - 1),
                        )
                    cvs = vcsp.tile([P, 64], f16, tag="cvs", name=f"cv{b}{gt_}")
                    svs = vcsp.tile([P, 64], f16, tag="svs", name=f"sv{b}{gt_}")
                    nc.vector.tensor_scalar_mul(
                        cvs, cosv[:, gt_, :], rsqa[b][:, gt_ : gt_ + 1]
                    )
                    nc.vector.tensor_scalar_mul(
                        svs, sinv[:, gt_, :], rsqa[b][:, gt_ : gt_ + 1]
                    )
                    cvb = cvs[:, None, :].to_broadcast([P, HPC, 64])
                    svb = svs[:, None, :].to_broadcast([P, HPC, 64])
                    v1 = vtmp.tile([P, HPC, 64], f16, tag="v1", name=f"v1_{b}{gt_}")
                    v2 = vtmp.tile([P, HPC, 64], f16, tag="v2", name=f"v2_{b}{gt_}")
                    v3 = vtmp.tile([P, HPC, 64], f16, tag="v3", name=f"v3_{b}{gt_}")
                    v4 = vtmp.tile([P, HPC, 64], f16, tag="v4", name=f"v4_{b}{gt_}")
                    nc.vector.tensor_mul(v1, vp_[:, :, 0:64], cvb)
                    nc.vector.tensor_mul(v2, vp_[:, :, 64:P], svb)
                    nc.vector.tensor_tensor(
                        VV[b][:, gt_, :, 0:64], v1, v2, OP.subtract
                    )
                    nc.vector.tensor_mul(v3, vp_[:, :, 0:64], svb)
                    nc.vector.tensor_mul(v4, vp_[:, :, 64:P], cvb)
                    nc.vector.tensor_tensor(VV[b][:, gt_, :, 64:P], v3, v4, OP.add)

            # causal attention, transposed orientation: AVT[dh, q].
            # Software-pipelined: the scores MM for chunk kc+1 issues
            # before the AV MM for chunk kc. Denominator: DVE running sum
            # of the exp tiles, one ones-matmul per chain. The whole
            # normalization tail (dn MM, recip, st mul, DMA) is deferred
            # 1-3 score-chunks into the NEXT chain so the in-order PE/DVE
            # queues never block on the cross-engine tail.
            pend = [None]
            last_st = [None]
            tok_st = [None]

            def make_attn(exps, exsp, rdBp, stg, scps, avps, dnps):
                def flush_tail():
                    # denominator matmul: ones_sq as lhsT makes the column sum
                    # land pre-broadcast on all 128 partitions, so the recip is
                    # a full-width (fast) DVE op and gpsimd stays out of it
                    if pend[0] is None or pend[0][0] != "s1":
                        return
                    _, fb, fh, fqc, exsum, avp_ = pend[0]
                    dnp_ = dnps.tile([P, TB], f32, tag="dnm", name=f"dm{fb}{fh}{fqc}")
                    nc.tensor.matmul(dnp_, ones_sq, exsum, start=True, stop=True)
                    pend[0] = ("s2", fb, fh, fqc, avp_, dnp_)

                def flush_tail1b():
                    if pend[0] is None or pend[0][0] != "s2":
                        return
                    _, fb, fh, fqc, avp_, dnp_ = pend[0]
                    rdB_ = rdBp.tile(
                        [P, TB], f32, tag="rdB", name=f"rB2{fb}{fh}{fqc}"
                    )
                    nc.vector.reciprocal_approx_fast(rdB_, dnp_)
                    pend[0] = ("s3", fb, fh, fqc, avp_, rdB_)

                def flush_tail2():
                    if pend[0] is None or pend[0][0] != "s3":
                        return
                    _, fb, fh, fqc, avp_, rdB_ = pend[0]
                    st = stg.tile(
                        [P, TB], f16, tag=f"stage{fh}", name=f"stage{fb}{fh}{fqc}"
                    )
                    nc.vector.tensor_mul(st, avp_, rdB_)
                    last_st[0] = st
                    if (fb, fh, fqc) == (0, 1, 3):
                        tok_st[0] = st
                    if fh == 0:
                        nc.sync.dma_start(a2a_in0[fb * 4 + fqc], st)
                    else:
                        nc.sync.dma_start(a2a_in1[fb * 4 + fqc], st)
                    if fqc == 3 and fb == B - 1:
                        nc.gpsimd.collective_compute(
                            "AllToAll",
                            mybir.AluOpType.bypass,
                            replica_groups=[list(range(NC))],
                            ins=[(a2a_in0 if fh == 0 else a2a_in1).opt()],
                            outs=[(a2a_out0 if fh == 0 else a2a_out1).opt()],
                        )
                    pend[0] = None

                def emit_attn(b, h):
                    # score chunks processed in PAIRS: two sc matmuls land in
                    # the two adjacent PSUM banks of one [P,2,TB] tile, ONE
                    # exp activation covers both (halves the scalar-engine
                    # per-op overhead that paces the long chains), and the
                    # denominator partial sums accumulate pairwise on DVE.
                    # The av matmuls for pair p-1 are emitted after the sc
                    # pair p so exp latency stays hidden (PE in-order).
                    for qc in range(4):
                        qslc = slice(qc * TB, (qc + 1) * TB)
                        avp_ = avps.tile(
                            [P, TB], f32, tag="av", name=f"av{b}{h}{qc}"
                        )
                        nkc = 4 * qc + 4
                        npr = nkc // 2
                        exsum2 = exsp.tile(
                            [P, 2, TB], f16, tag="exs", name=f"exs{b}{h}{qc}"
                        )
                        exsum = exsp.tile(
                            [P, TB], f16, tag="exsf", name=f"exsf{b}{h}{qc}"
                        )
                        scpair = {}

                        def emit_body(p):
                            scp2 = scpair.pop(p)
                            ex2 = exps.tile(
                                [P, 2, TB], f16, tag="ex", name=f"ex{b}{h}{qc}_{p}"
                            )
                            nc.scalar.activation(
                                ex2, scp2, AF.Exp, scale=inv_sqrt_dh
                            )
                            for j in range(2):
                                kc = 2 * p + j
                                if kc >= 4 * qc:
                                    nc.vector.tensor_mul(
                                        ex2[:, j, :], ex2[:, j, :],
                                        maskt[:, kc - 4 * qc, :],
                                    )
                            for j in range(2):
                                kc = 2 * p + j
                                nc.tensor.matmul(
                                    avp_,
                                    VV[b][:, kc, h, :],
                                    ex2[:, j, :],
                                    start=(kc == 0),
                                    stop=(kc == nkc - 1),
                                )
                            if p == 0:
                                nc.vector.tensor_copy(exsum2, ex2)
                            else:
                                nc.vector.tensor_tensor(exsum2, exsum2, ex2, OP.add)

                        for p in range(npr):
                            scp2 = scps.tile(
                                [P, 2, TB], f32, tag="sc", name=f"sc{b}{h}{qc}_{p}"
                            )
                            scpair[p] = scp2
                            for j in range(2):
                                kc = 2 * p + j
                                nc.tensor.matmul(
                                    scp2[:, j, :],
                                    KK[b, h][:, kc * P : (kc + 1) * P],
                                    QR[b, h][:, qslc],
                                    start=True,
                                    stop=True,
                                )
                            if p >= 1:
                                emit_body(p - 1)
                            if p == 1:
                                flush_tail()
                            elif p == 2:
                                flush_tail1b()
                            elif p == 3:
                                flush_tail2()
                        emit_body(npr - 1)
                        nc.vector.tensor_tensor(
                            exsum, exsum2[:, 0, :], exsum2[:, 1, :], OP.add
                        )
                        flush_tail1b()
                        flush_tail2()
                        pend[0] = ("s1", b, h, qc, exsum, avp_)

                def finish():
                    flush_tail()
                    flush_tail1b()
                    flush_tail2()

                return emit_attn, finish

            # emission order: startup-critical DMAs (x rows for stats, then
            # wq/rope tables, then the first xT block) go first; b1 stats
            # pipeline under b0's projections so nothing stalls at the batch
            # boundary
            for b in range(B):
                for h in range(HPC):
                    QR[b, h] = qrk.tile(
                        [P, S], f16, tag=f"qr{b}{h}", name=f"qr{b}_{h}"
                    )
                    KK[b, h] = qrk.tile(
                        [P, S], f16, tag=f"kk{b}{h}", name=f"kk{b}_{h}"
                    )
                VV[b] = vsb.tile(
                    [P, KC, HPC, P], f16, tag=f"v{b}", name=f"vv{b}"
                )
            emit_stats(0, 0)
            nc.sync.dma_start(wq_sb, wq.rearrange("(c p) m -> p c m", p=P))
            nc.sync.dma_start(cosq, cos_qt.ap())
            nc.sync.dma_start(sinq, sin_qt.ap())
            xT00 = load_xT(0, 0)
            nc.sync.dma_start(wk_sb, wk.rearrange("(c p) m -> p c m", p=P))
            emit_stats(0, 1)
            xT01 = load_xT(0, 1)
            nc.sync.dma_start(wv_sb, wv.rearrange("(c p) m -> p c m", p=P))
            nc.sync.dma_start(cosv, cos_v.rearrange("(i p) f -> p i f", p=P))
            nc.sync.dma_start(sinv, sin_v.rearrange("(i p) f -> p i f", p=P))
            nc.sync.dma_start(maskt, masks.rearrange("m p t -> p m t"))
            emit_proj(0, 0, xT00)
            nc.sync.dma_start(b1_sb, b1s.ap())
            emit_stats(0, 2)
            xT02 = load_xT(0, 2)
            emit_proj(0, 1, xT01)
            emit_stats(0, 3)
            xT03 = load_xT(0, 3)
            emit_proj(0, 2, xT02)
            emit_stats(1, 0)
            emit_proj(0, 3, xT03)
            for qc in range(4):
                if qc < 3:
                    emit_stats(1, qc + 1)
                emit_proj(1, qc)
            # W_O pass-A weights: no dependencies, land during attention at
            # full HBM bandwidth (before the collectives run)
            nc.sync.dma_start(woeAt, wo.rearrange("(c p) e -> p c e", p=P)[:, 0:8, :])
            pj.close()

            # ---------- attention, head-major: h0 for both batches first so
            # the h=0 AllToAll fires at ~50% of attention and hides under the
            # h=1 chains; only the h=1 AllToAll tail is exposed (covered by
            # W_O pass A) ----------
            with ExitStack() as pa:
                exps = pa.enter_context(tc.tile_pool(name="exps", bufs=6))
                exsp = pa.enter_context(tc.tile_pool(name="exsp", bufs=2))
                rdBp = pa.enter_context(tc.tile_pool(name="rdBp", bufs=2))
                stg = pa.enter_context(tc.tile_pool(name="stg", bufs=3))
                scps = pa.enter_context(
                    tc.tile_pool(name="scps", bufs=2, space="PSUM")
                )
                avps = pa.enter_context(
                    tc.tile_pool(name="avps", bufs=3, space="PSUM")
                )
                dnps = pa.enter_context(
                    tc.tile_pool(name="dnps", bufs=1, space="PSUM")
                )
                emit_attn, attn_finish = make_attn(
                    exps, exsp, rdBp, stg, scps, avps, dnps
                )
                emit_attn(0, 0)
                emit_attn(1, 0)
                emit_attn(0, 1)
                emit_attn(1, 1)
                attn_finish()
                # token copy: makes the hoTe DMA region-dependent on the LAST
                # attention staging tile, so the scheduler cannot float its
                # doorbell (which waits on the h0 AllToAll) ahead of the
                # remaining attention work on the scalar queue
                nc.vector.tensor_copy(
                    hoTe[0:1, 0, 0:2], (tok_st[0] or last_st[0])[0:1, 0:2]
                )

        # ---------- phases 3+4 (transposed): W_O + residual + rmsnorm2 + MLP
        h2Tt = h2Tp.tile([P, KC, TB], f16)
        with ExitStack() as p34:
            x2p = p34.enter_context(tc.tile_pool(name="x2p", bufs=1))
            scr2 = p34.enter_context(tc.tile_pool(name="scr2", bufs=3))
            sm2 = p34.enter_context(tc.tile_pool(name="sm2", bufs=1))
            b2p = p34.enter_context(tc.tile_pool(name="b2p", bufs=1))
            outp = p34.enter_context(tc.tile_pool(name="outp", bufs=2))
            wops = p34.enter_context(tc.tile_pool(name="wops", bufs=3, space="PSUM"))
            ssps = p34.enter_context(tc.tile_pool(name="ssps", bufs=1, space="PSUM"))

            x2Tt = x2p.tile([P, KC, TB], f16)
            b2T = b2p.tile([P, KC], f32)
            nc.sync.dma_start(b2T, b2s.ap())
            wov = wo.rearrange("(c p) e -> p c e", p=P)
            with ExitStack() as p3:
                hoTp = p3.enter_context(tc.tile_pool(name="hoT", bufs=1))
                xrTp = p3.enter_context(tc.tile_pool(name="xrT", bufs=1))
                woep = p3.enter_context(tc.tile_pool(name="woe", bufs=1))

                # hoT staging rides the (idle) scalar engine's DMA queue so
                # its wait on the collectives can't block the a2a staging or
                # weight-prefetch DMAs on the sync queue
                nc.scalar.dma_start(hoTe, a2a_out0.rearrange("n p t -> p n t"))
                hoTo = hoTp.tile([P, 8, TB], f16)
                nc.scalar.dma_start(hoTo, a2a_out1.rearrange("n p t -> p n t"))
                xrT = xrTp.tile([P, KC, TB], f16)
                nc.sync.dma_start(xrT, xrT_d.rearrange("(c p) t -> p c t", p=P))
                woeBt = woep.tile([P, 8, D], f16)
                nc.sync.dma_start(woeBt, wov[:, 8:16, :])
                # pass A: even-head d-chunks (weights + chunks preloaded);
                # runs under the h=1 AllToAll tail
                for e in range(KC):
                    wp = wops.tile([P, TB], f32, tag="wo", name=f"woA{e}")
                    for j in range(8):
                        nc.tensor.matmul(
                            wp,
                            woeAt[:, j, e * P : (e + 1) * P],
                            hoTe[:, j, :],
                            start=(j == 0),
                            stop=(j == 7),
                        )
                    nc.vector.tensor_tensor(
                        x2Tt[:, e, :], wp, xrT[:, e, :], OP.add
                    )
                # pass B: odd-head d-chunks (h=1 AllToAll); the rmsnorm2
                # square/column-reduce chain trails one e-chunk behind so the
                # ones-matmul never stalls the in-order PE queue
                ssq2 = ssps.tile([P, TB], f32)

                def emit_sq(e):
                    s2 = scr2.tile([P, TB], f16, tag="s2", name=f"s2_{e}")
                    nc.scalar.activation(s2, x2Tt[:, e, :], AF.Square)
                    nc.tensor.matmul(
                        ssq2, ones_sq, s2, start=(e == 0), stop=(e == KC - 1)
                    )

                for e in range(KC):
                    wp = wops.tile([P, TB], f32, tag="wo", name=f"woB{e}")
                    for j in range(8):
                        nc.tensor.matmul(
                            wp,
                            woeBt[:, j, e * P : (e + 1) * P],
                            hoTo[:, j, :],
                            start=(j == 0),
                            stop=(j == 7),
                        )
                    nc.vector.tensor_tensor(
                        x2Tt[:, e, :], x2Tt[:, e, :], wp, OP.add
                    )
                    if e >= 1:
                        emit_sq(e - 1)
                emit_sq(KC - 1)
                rms2 = sm2.tile([P, TB], f32, tag="rms2")
                nc.scalar.activation(
                    rms2, ssq2, AF.Sqrt, bias=eps_t, scale=float(1.0 / D)
                )
                rdB2 = sm2.tile([P, TB], f32, tag="rdB2")
                nc.vector.reciprocal_approx_fast(rdB2, rms2)
                rdB2b = rdB2[:, None, :].to_broadcast([P, 8, TB])
                nc.vector.tensor_mul(h2Tt[:, 0:8, :], x2Tt[:, 0:8, :], rdB2b)
                for e in range(8, KC):
                    nc.gpsimd.tensor_mul(
                        h2Tt[:, e, :], x2Tt[:, e, :], rdB2
                    )
                # fold B2 into x2 AFTER h2 is derived (out = x2 + B2 + mlp)
                for e in range(KC):
                    nc.vector.tensor_scalar_add(
                        x2Tt[:, e, :], x2Tt[:, e, :], b2T[:, e : e + 1]
                    )

            # ---------- MLP1 ----------
            # gtt opens after phase 3 closes so it can reuse that SBUF region
            gtp = p34.enter_context(tc.tile_pool(name="gtp", bufs=1))
            gtt = gtp.tile([P, FC, TB], f16)
            # w2p opens before w1p so its region is fresh: the first MLP2
            # weight DMAs don't carry a WAR dependency on the last w1 reads
            w2p = p34.enter_context(tc.tile_pool(name="w2p", bufs=3))
            m1ctx = ExitStack()
            w1p = m1ctx.enter_context(tc.tile_pool(name="w1p", bufs=2))
            m1ps = m1ctx.enter_context(tc.tile_pool(name="m1ps", bufs=3, space="PSUM"))
            for fg in range(16):
                w1t = w1p.tile([P, KC, TB], f16, tag="w1", name=f"w1_{fg}")
                nc.sync.dma_start(w1t, w1r.ap()[fg])
                for fs in range(4):
                    f = fg * 4 + fs
                    mp = m1ps.tile([P, TB], f32, tag="m1", name=f"m1_{f}")
                    for d in range(KC):
                        nc.tensor.matmul(
                            mp,
                            w1t[:, d, fs * P : (fs + 1) * P],
                            h2Tt[:, d, :],
                            start=(d == 0),
                            stop=(d == KC - 1),
                        )
                    nc.scalar.activation(
                        gtt[:, f, :], mp, AF.Relu, bias=b1_sb[:, f : f + 1]
                    )
            m1ctx.close()

            # ---------- MLP2 (transposed output) ----------
            m2ctx = ExitStack()
            m2ps = m2ctx.enter_context(tc.tile_pool(name="m2ps", bufs=2, space="PSUM"))
            outv = out_d.rearrange("(c p) t -> p c t", p=P)
            for e in range(KC):
                w2a = w2p.tile([P, 32, P], f16, tag="w2", name=f"w2a_{e}")
                nc.sync.dma_start(
                    w2a, w2r.ap()[e][:, 0:32, :]
                )
                w2b = w2p.tile([P, 32, P], f16, tag="w2", name=f"w2b_{e}")
                nc.sync.dma_start(
                    w2b, w2r.ap()[e][:, 32:64, :]
                )
                mT = m2ps.tile([P, TB], f32, tag="m2", name=f"m2_{e}")
                for f in range(FC):
                    w2t = w2a if f < 32 else w2b
                    nc.tensor.matmul(
                        mT,
                        w2t[:, f % 32, :],
                        gtt[:, f, :],
                        start=(f == 0),
                        stop=(f == FC - 1),
                    )
                outt = outp.tile([P, TB], f32, tag="out", name=f"out{e}")
                nc.vector.tensor_tensor(outt, mT, x2Tt[:, e, :], OP.add)
                nc.sync.dma_start(outv[:, e, :], outt)
            m2ctx.close()

    nc.compile()
    return nc


def _host_inputs(inputs):
    x = np.asarray(inputs["x"], np.float32)
    Wq = np.asarray(inputs["Wq"], np.float32)
    Wk = np.asarray(inputs["Wk"], np.float32)
    Wv = np.asarray(inputs["Wv"], np.float32)
    W_O = np.asarray(inputs["W_O"], np.float32)
    scale1 = np.asarray(inputs["scale1"], np.float32)
    scale2 = np.asarray(inputs["scale2"], np.float32)
    W1 = np.asarray(inputs["W1"], np.float32)
    B1 = np.asarray(inputs["B1"], np.float32)
    W2 = np.asarray(inputs["W2"], np.float32)
    B2 = np.asarray(inputs["B2"], np.float32)

    perm = np.concatenate([np.arange(0, DH, 2), np.arange(1, DH, 2)])
    # fold rmsnorm scales into the following matmuls
    Wq_s = Wq * scale1[None, :, None]
    Wk_s = Wk * scale1[None, :, None]
    Wv_s = Wv * scale1[None, :, None]
    W1_s = W1 * scale2[:, None]

    # W_O rows reordered to match the permuted, head-major layout of HO.T,
    # with the h=0 (even) heads' row blocks first, then the h=1 (odd) heads
    # (matching the a2a_out0 / a2a_out1 arrival order)
    head_order = list(range(0, H, 2)) + list(range(1, H, 2))
    row_order = np.concatenate([h * DH + perm for h in head_order])
    wo_c = np.ascontiguousarray(W_O[row_order, :]).astype(np.float16)
    w1_c = W1_s.astype(np.float16)
    w2_c = W2.astype(np.float16)
    b1s_c = np.ascontiguousarray(B1.reshape(FC, P).T)
    b2s_c = np.ascontiguousarray(B2.reshape(KC, P).T)

    xflat = x.reshape(B * S, D)
    xf_c = xflat.astype(np.float16)
    xT_c = np.ascontiguousarray(xflat.T).astype(np.float16)
    # block-sliced xT slabs: full-length DMA lines (16KB vs 1KB)
    xT3_c = np.ascontiguousarray(
        xT_c.reshape(KC, P, B * S)
        .reshape(KC, P, B * 4, TB)
        .transpose(2, 1, 0, 3)
    )
    w1r_c = np.ascontiguousarray(
        w1_c.reshape(KC, P, 16, TB).transpose(2, 1, 0, 3)
    )
    w2r_c = np.ascontiguousarray(
        w2_c.reshape(FC, P, KC, P).transpose(2, 1, 0, 3)
    )

    # rope tables
    pos = np.arange(S, dtype=np.float64)
    pidx = np.arange(64, dtype=np.float64)
    theta_p = 1.0 / THETA ** (2.0 * pidx / DH)
    ang = pos[None, :] * theta_p[:, None]  # [64, S]
    cos_qt_c = np.cos(ang).astype(np.float16)
    sin_qt_c = np.sin(ang).astype(np.float16)
    cos_v_c = np.ascontiguousarray(cos_qt_c.T)
    sin_v_c = np.ascontiguousarray(sin_qt_c.T)

    ii = np.arange(P)[:, None]
    jj = np.arange(TB)[None, :]
    masks_c = np.stack(
        [(ii + P * m <= jj).astype(np.float16) for m in range(4)]
    )

    in_maps = []
    for c in range(NC):
        b, r = c // 4, c % 4
        heads = [HPC * c, HPC * c + 1]
        wq_c = np.concatenate([Wq_s[h][:, perm] for h in heads], 1).astype(np.float16)
        wk_c = np.concatenate([Wk_s[h][:, perm] for h in heads], 1).astype(np.float16)
        wv_c = np.concatenate([Wv_s[h][:, perm] for h in heads], 1).astype(np.float16)
        tok0 = b * S + r * TB
        in_maps.append(
            {
                "xf": xf_c,
                "xrT": np.ascontiguousarray(xT_c[:, tok0 : tok0 + TB]),
                "wq": np.ascontiguousarray(wq_c),
                "wk": np.ascontiguousarray(wk_c),
                "wv": np.ascontiguousarray(wv_c),
                "wo": wo_c,
                "xT3": xT3_c,
                "w1r": w1r_c,
                "w2r": w2r_c,
                "b1s": b1s_c,
                "b2s": b2s_c,
                "cos_qt": cos_qt_c,
                "sin_qt": sin_qt_c,
                "cos_v": cos_v_c,
                "sin_v": sin_v_c,
                "masks": masks_c,
            }
        )
    return in_maps


def kernel(**inputs):
    from concourse.bass_utils import run_bass_kernel_spmd

    trace = bool(os.environ.get("BASS_KERNEL_TRACE"))
    if trace:
        _install_ntff_hook()

    if "nc" not in _CACHE:
        _CACHE["nc"] = _build()
    nc = _CACHE["nc"]

    in_maps = _host_inputs(inputs)
    r = run_bass_kernel_spmd(nc, in_maps, list(range(NC)), trace=trace)
    kernel.last_exec_time_ns = r.exec_time_ns

    out = np.empty((B, S, D), np.float32)
    for c in range(NC):
        b, rr = c // 4, c % 4
        out[b, rr * TB : (rr + 1) * TB, :] = r.results[c]["out"].T
    return out


kernel.last_exec_time_ns = None


# revision 42
# speedup vs baseline: 1.0032x; 1.0032x over previous
"""Decoder block (rmsnorm->MHA(rope on Q,V)->W_O residual->rmsnorm->MLP residual)
on 8 Trainium2 NeuronCores.

Sharding: each core computes attention for 2 of the 16 heads over BOTH batches
(weights sharded by head), then AllToAll redistributes head outputs so each
core owns one (batch, 512-token-block) slice for the W_O projection, second
rmsnorm and MLP (full weights, token-sharded). Host concatenates the 8
token-block outputs (transposing each [D,TB] slab).

v4 structure (1258us -> ~1005us on HW):
- attention is HEAD-MAJOR: h=0 chains for both batches run first so the h=0
  AllToAll fires at ~50% of attention and hides under the h=1 chains; the
  h=1 AllToAll tail is covered by W_O pass A (even-head chunks, weights
  preloaded during attention; odd-head pass B after).
- score chunks processed in pairs: two score matmuls land in adjacent PSUM
  banks of one [P,2,TB] tile and ONE exp activation covers both, halving the
  scalar-engine overhead that paces the long causal chains.
- softmax denominator: DVE pairwise running sum of the exp tiles + one
  ones_sq-matmul per chain whose output lands PRE-BROADCAST on all 128
  partitions, so the reciprocal is a fast full-width approx op and gpsimd
  stays off the critical path. The whole tail is deferred into the next
  chain so the in-order PE/DVE queues never block on it.
- phases 3/4 fully transposed: W_O emits x2^T directly (lhsT=W_O tile,
  rhs=head-outputs), rmsnorm2 reduces over partitions via a ones_sq-matmul,
  MLP2 emits out^T; no PE transposes, no x2 DRAM roundtrip.
- the post-collective staging DMAs ride the scalar queue, gated by a token
  copy off the last attention tile so the scheduler cannot float their
  semaphore wait ahead of remaining attention work.
- xT / W1 / W2 are host-relayouted into per-block slabs so every DMA line is
  16KB (full HBM efficiency) instead of 1KB/256B strided reads.
- startup: x-row DMAs + stats first, weights behind them; warmup matmuls are
  N=512 and span the initial DMA window so HAM stays warm.
"""

import os

import numpy as np

B, S, D, H = 2, 2048, 2048, 16
DH = 128
NC = 8
HPC = 2  # heads per core
P = 128
TB = 512  # token block (= S/4) and q-chunk width
KC = D // P  # 16 contraction chunks over D
FC = (4 * D) // P  # 64 contraction chunks over the MLP hidden dim
EPS = 1e-8
THETA = 10000.0

_CACHE = {}


def _install_ntff_hook():
    """Optional: register the axon NTFF profiling hook so trace=True works."""
    import sys
    import types

    if "antenv.axon_hooks" in sys.modules:
        return True
    try:
        mod = types.ModuleType("antenv.axon_hooks")
        _hook = [None]
        mod.set_axon_ntff_profile_hook = lambda h: _hook.__setitem__(0, h)
        mod.get_axon_ntff_profile_hook = lambda: _hook[0]
        import antenv
        from trn_agent_boot.trn_boot import _ntff_profile_via_ctypes

        sys.modules["antenv.axon_hooks"] = mod
        antenv.axon_hooks = mod
        mod.set_axon_ntff_profile_hook(
            _ntff_profile_via_ctypes("/opt/axon/libaxon_pjrt.so")
        )
        return True
    except Exception:
        return False


def _build():
    import concourse.bass as bass
    import concourse.mybir as mybir
    import concourse.tile as tile
    from concourse import bacc
    from concourse.masks import make_identity
    from contextlib import ExitStack

    f32 = mybir.dt.float32
    f16 = mybir.dt.float16
    AF = mybir.ActivationFunctionType
    OP = mybir.AluOpType

    nc = bacc.Bacc("TRN2", target_bir_lowering=False, debug=False, num_devices=NC)

    xT3_d = nc.dram_tensor("xT3", [B * 4, P, KC, TB], f16, kind="ExternalInput")
    xf_d = nc.dram_tensor("xf", [B * S, D], f16, kind="ExternalInput")
    xrT_d = nc.dram_tensor("xrT", [D, TB], f16, kind="ExternalInput")
    wq = nc.dram_tensor("wq", [D, HPC * P], f16, kind="ExternalInput")
    wk = nc.dram_tensor("wk", [D, HPC * P], f16, kind="ExternalInput")
    wv = nc.dram_tensor("wv", [D, HPC * P], f16, kind="ExternalInput")
    wo = nc.dram_tensor("wo", [D, D], f16, kind="ExternalInput")
    w1r = nc.dram_tensor("w1r", [16, P, KC, TB], f16, kind="ExternalInput")
    w2r = nc.dram_tensor("w2r", [KC, P, FC, P], f16, kind="ExternalInput")
    b1s = nc.dram_tensor("b1s", [P, FC], f32, kind="ExternalInput")
    b2s = nc.dram_tensor("b2s", [P, KC], f32, kind="ExternalInput")
    cos_qt = nc.dram_tensor("cos_qt", [64, S], f16, kind="ExternalInput")
    sin_qt = nc.dram_tensor("sin_qt", [64, S], f16, kind="ExternalInput")
    cos_v = nc.dram_tensor("cos_v", [S, 64], f16, kind="ExternalInput")
    sin_v = nc.dram_tensor("sin_v", [S, 64], f16, kind="ExternalInput")
    masks = nc.dram_tensor("masks", [4, P, TB], f16, kind="ExternalInput")
    out_d = nc.dram_tensor("out", [D, TB], f32, kind="ExternalOutput")

    inv_sqrt_dh = float(1.0 / np.sqrt(DH))

    with tile.TileContext(nc) as tc, ExitStack() as ctx:
        cst = ctx.enter_context(tc.tile_pool(name="cst", bufs=1))
        dram = ctx.enter_context(tc.tile_pool(name="dram", bufs=1, space="DRAM"))
        # long-lived across phases 3-4
        h2Tp = ctx.enter_context(tc.tile_pool(name="h2Tp", bufs=1))

        eps_t = cst.tile([P, 1], f32)
        nc.vector.memset(eps_t, EPS)
        ident16 = cst.tile([P, P], f16)
        make_identity(nc, ident16)
        ones_sq = cst.tile([P, P], f16)
        nc.vector.memset(ones_sq, 1.0)
        warm_rhs = cst.tile([P, TB], f16)
        nc.vector.memset(warm_rhs, 0.0)
        b1_sb = cst.tile([P, FC], f32)
        # pass-A phase-3 inputs, hoisted so their DMAs can run during
        # attention (no SBUF region reuse -> no false WAR dependencies)
        woeAt = cst.tile([P, 8, D], f16)
        hoTe = cst.tile([P, 8, TB], f16)
        with tc.tile_pool(name="wrm", bufs=1, space="PSUM") as wrmp:
            wrm = wrmp.tile([P, TB], f32)
            for _ in range(85):
                nc.tensor.matmul(wrm, ident16, warm_rhs, start=True, stop=True)

        # internal DRAM for the collectives
        a2a_in0 = dram.tile([NC, P, TB], f16, name="a2a_in0")
        a2a_out0 = dram.tile([NC, P, TB], f16, name="a2a_out0")
        a2a_in1 = dram.tile([NC, P, TB], f16, name="a2a_in1")
        a2a_out1 = dram.tile([NC, P, TB], f16, name="a2a_out1")

        # ---------- phase 1+2: rmsnorm1 fused with QKV/attention ----------
        with ExitStack() as p2:
            # spanning pools: Q/K/V for BOTH batches (head-major attention
            # order needs them concurrently) + the attention mask
            qrk = p2.enter_context(tc.tile_pool(name="qrk", bufs=1))
            vsb = p2.enter_context(tc.tile_pool(name="vsb", bufs=1))
            mskp = p2.enter_context(tc.tile_pool(name="mskp", bufs=1))
            maskt = mskp.tile([P, 4, TB], f16)

            pj = ExitStack()
            xTp = pj.enter_context(tc.tile_pool(name="xTp", bufs=2))
            xfp = pj.enter_context(tc.tile_pool(name="xfp", bufs=2))
            smp = pj.enter_context(tc.tile_pool(name="smp", bufs=4))
            rsqp = pj.enter_context(tc.tile_pool(name="rsqp", bufs=1))
            diagp = pj.enter_context(tc.tile_pool(name="diagp", bufs=2))
            cqsp = pj.enter_context(tc.tile_pool(name="cqsp", bufs=2))
            vcsp = pj.enter_context(tc.tile_pool(name="vcsp", bufs=2))
            acst = pj.enter_context(tc.tile_pool(name="acst", bufs=1))
            rtmp = pj.enter_context(tc.tile_pool(name="rtmp", bufs=1))
            vtmp = pj.enter_context(tc.tile_pool(name="vtmp", bufs=1))
            qkps = pj.enter_context(tc.tile_pool(name="qkps", bufs=2, space="PSUM"))
            vps = pj.enter_context(tc.tile_pool(name="vps", bufs=1, space="PSUM"))

            # tile declarations for the deferred weight/table DMAs (emitted
            # after the first stats DMAs so the startup path drains first)
            wq_sb = acst.tile([P, KC, HPC * P], f16)
            wk_sb = acst.tile([P, KC, HPC * P], f16)
            wv_sb = acst.tile([P, KC, HPC * P], f16)
            cosq = acst.tile([64, S], f16)
            sinq = acst.tile([64, S], f16)
            cosv = acst.tile([P, KC, 64], f16)
            sinv = acst.tile([P, KC, 64], f16)

            rsqa = {}
            for b in range(B):
                rsqa[b] = rsqp.tile([P, KC], f32, tag=f"rsq{b}", name=f"rsq{b}")
            diags = {}
            QR = {}
            KK = {}
            VV = {}

            def emit_stats(b, qc):
                # rms stats for the 4 token-chunks of block (b, qc); Squares
                # batched before Sqrt/recip so the activation table loads once
                diag = diagp.tile([P, 4, P], f16, tag=f"dg{b}", name=f"dg{b}_{qc}")
                ssqB = smp.tile([P, 4], f32, tag="ssq", name=f"ssq{b}_{qc}")
                for i in range(4):
                    g = qc * 4 + i
                    xfr = xfp.tile([P, D], f16, tag="xf", name=f"xf{b}_{g}")
                    nc.scalar.dma_start(
                        xfr, xf_d.ap()[b * S + g * P : b * S + (g + 1) * P, :]
                    )
                    nc.scalar.activation(
                        xfr, xfr, AF.Square, accum_out=ssqB[:, i : i + 1]
                    )
                rmsB = smp.tile([P, 4], f32, tag="rms", name=f"rms{b}_{qc}")
                nc.scalar.activation(
                    rmsB, ssqB, AF.Sqrt, bias=eps_t, scale=float(1.0 / D)
                )
                nc.vector.reciprocal(rsqa[b][:, qc * 4 : (qc + 1) * 4], rmsB)
                for i in range(4):
                    g = qc * 4 + i
                    nc.vector.tensor_scalar_mul(
                        diag[:, i, :], ident16, rsqa[b][:, g : g + 1]
                    )
                diags[(b, qc)] = diag

            def load_xT(b, qc):
                xTt = xTp.tile([P, KC, TB], f16, tag="xT", name=f"xT{b}_{qc}")
                nc.sync.dma_start(xTt, xT3_d.ap()[b * 4 + qc])
                return xTt

            def emit_proj(b, qc, xTt=None):
                if xTt is None:
                    xTt = load_xT(b, qc)
                # rsqB[p, q] = rsq per token q, on all partitions p
                rsqB = qkps.tile([P, TB], f32, tag="qk", name=f"rB{b}_{qc}")
                nc.tensor.matmul(
                    rsqB,
                    ones_sq,
                    diags.pop((b, qc)).rearrange("p a b -> p (a b)"),
                    start=True,
                    stop=True,
                )
                qslc = slice(qc * TB, (qc + 1) * TB)
                rsqBs = cqsp.tile([P, TB], f16, tag="rBs", name=f"rBs{b}_{qc}")
                nc.vector.tensor_copy(rsqBs, rsqB)
                cqs = cqsp.tile([64, TB], f16, tag="cqs", name=f"cqs{b}_{qc}")
                sqs = cqsp.tile([64, TB], f16, tag="sqs", name=f"sqs{b}_{qc}")
                nc.vector.tensor_mul(cqs, cosq[:, qslc], rsqBs[0:64, :])
                nc.vector.tensor_mul(sqs, sinq[:, qslc], rsqBs[0:64, :])

                for h in range(HPC):
                    # Q projection + rope (even dims 0:64 = x1, odd = x2);
                    # rmsnorm scale folded into cqs/sqs
                    qp = qkps.tile([P, TB], f32, tag="qk", name=f"qp{b}{qc}{h}")
                    for d in range(KC):
                        nc.tensor.matmul(
                            qp,
                            wq_sb[:, d, h * P : (h + 1) * P],
                            xTt[:, d, :],
                            start=(d == 0),
                            stop=(d == KC - 1),
                        )
                    t1 = rtmp.tile([64, TB], f16, tag="t1", name=f"t1_{b}{qc}{h}")
                    t2 = rtmp.tile([64, TB], f16, tag="t2", name=f"t2_{b}{qc}{h}")
                    t3 = rtmp.tile([64, TB], f16, tag="t3", name=f"t3_{b}{qc}{h}")
                    t4 = rtmp.tile([64, TB], f16, tag="t4", name=f"t4_{b}{qc}{h}")
                    nc.vector.tensor_mul(t1, qp[0:64, :], cqs)
                    nc.vector.tensor_mul(t2, qp[64:P, :], sqs)
                    nc.vector.tensor_tensor(QR[b, h][0:64, qslc], t1, t2, OP.subtract)
                    nc.vector.tensor_mul(t3, qp[0:64, :], sqs)
                    nc.vector.tensor_mul(t4, qp[64:P, :], cqs)
                    nc.vector.tensor_tensor(QR[b, h][64:P, qslc], t3, t4, OP.add)
                    # K projection: raw copy; rmsnorm scale rides exp()
                    kp = qkps.tile([P, TB], f32, tag="qk", name=f"kp{b}{qc}{h}")
                    for d in range(KC):
                        nc.tensor.matmul(
                            kp,
                            wk_sb[:, d, h * P : (h + 1) * P],
                            xTt[:, d, :],
                            start=(d == 0),
                            stop=(d == KC - 1),
                        )
                    # rmsnorm scale of the K side applied here (per token col)
                    nc.vector.tensor_mul(KK[b, h][:, qslc], kp, rsqBs)
                # V projection + rope, natural layout [tok, head, dh];
                # rmsnorm scale folded into the per-chunk rope tables
                for tt in range(4):
                    gt_ = qc * 4 + tt
                    vp_ = vps.tile([P, HPC, P], f32, tag="v", name=f"vp{b}_{qc}_{tt}")
                    for d in range(KC):
                        nc.tensor.matmul(
                            vp_.rearrange("p h k -> p (h k)"),
                            xTt[:, d, tt * P : (tt + 1) * P],
                            wv_sb[:, d, :],
                            start=(d == 0),
                            stop=(d == KC - 1),
                        )
                    cvs = vcsp.tile([P, 64], f16, tag="cvs", name=f"cv{b}{gt_}")
                    svs = vcsp.tile([P, 64], f16, tag="svs", name=f"sv{b}{gt_}")
                    nc.vector.tensor_scalar_mul(
                        cvs, cosv[:, gt_, :], rsqa[b][:, gt_ : gt_ + 1]
                    )
                    nc.vector.tensor_scalar_mul(
                        svs, sinv[:, gt_, :], rsqa[b][:, gt_ : gt_ + 1]
                    )
                    cvb = cvs[:, None, :].to_broadcast([P, HPC, 64])
                    svb = svs[:, None, :].to_broadcast([P, HPC, 64])
                    v1 = vtmp.tile([P, HPC, 64], f16, tag="v1", name=f"v1_{b}{gt_}")
                    v2 = vtmp.tile([P, HPC, 64], f16, tag="v2", name=f"v2_{b}{gt_}")
                    v3 = vtmp.tile([P, HPC, 64], f16, tag="v3", name=f"v3_{b}{gt_}")
                    v4 = vtmp.tile([P, HPC, 64], f16, tag="v4", name=f"v4_{b}{gt_}")
                    nc.vector.tensor_mul(v1, vp_[:, :, 0:64], cvb)
                    nc.vector.tensor_mul(v2, vp_[:, :, 64:P], svb)
                    nc.vector.tensor_tensor(
                        VV[b][:, gt_, :, 0:64], v1, v2, OP.subtract
                    )
                    nc.vector.tensor_mul(v3, vp_[:, :, 0:64], svb)
                    nc.vector.tensor_mul(v4, vp_[:, :, 64:P], cvb)
                    nc.vector.tensor_tensor(VV[b][:, gt_, :, 64:P], v3, v4, OP.add)

            # causal attention, transposed orientation: AVT[dh, q].
            # Software-pipelined: the scores MM for chunk kc+1 issues
            # before the AV MM for chunk kc. Denominator: DVE running sum
            # of the exp tiles, one ones-matmul per chain. The whole
            # normalization tail (dn MM, recip, st mul, DMA) is deferred
            # 1-3 score-chunks into the NEXT chain so the in-order PE/DVE
            # queues never block on the cross-engine tail.
            pend = [None]
            last_st = [None]
            tok_st = [None]

            def make_attn(exps, exsp, rdBp, stg, scps, avps, dnps):
                def flush_tail():
                    # denominator matmul: ones_sq as lhsT makes the column sum
                    # land pre-broadcast on all 128 partitions, so the recip is
                    # a full-width (fast) DVE op and gpsimd stays out of it
                    if pend[0] is None or pend[0][0] != "s1":
                        return
                    _, fb, fh, fqc, exsum, avp_ = pend[0]
                    dnp_ = dnps.tile([P, TB], f32, tag="dnm", name=f"dm{fb}{fh}{fqc}")
                    nc.tensor.matmul(dnp_, ones_sq, exsum, start=True, stop=True)
                    pend[0] = ("s2", fb, fh, fqc, avp_, dnp_)

                def flush_tail1b():
                    if pend[0] is None or pend[0][0] != "s2":
                        return
                    _, fb, fh, fqc, avp_, dnp_ = pend[0]
                    rdB_ = rdBp.tile(
                        [P, TB], f32, tag="rdB", name=f"rB2{fb}{fh}{fqc}"
                    )
                    nc.vector.reciprocal_approx_fast(rdB_, dnp_)
                    pend[0] = ("s3", fb, fh, fqc, avp_, rdB_)

                def flush_tail2():
                    if pend[0] is None or pend[0][0] != "s3":
                        return
                    _, fb, fh, fqc, avp_, rdB_ = pend[0]
                    st = stg.tile(
                        [P, TB], f16, tag=f"stage{fh}", name=f"stage{fb}{fh}{fqc}"
                    )
                    nc.vector.tensor_mul(st, avp_, rdB_)
                    last_st[0] = st
                    if (fb, fh, fqc) == (0, 1, 3):
                        tok_st[0] = st
                    if fh == 0:
                        nc.sync.dma_start(a2a_in0[fb * 4 + fqc], st)
                    else:
                        nc.sync.dma_start(a2a_in1[fb * 4 + fqc], st)
                    if fqc == 3 and fb == B - 1:
                        nc.gpsimd.collective_compute(
                            "AllToAll",
                            mybir.AluOpType.bypass,
                            replica_groups=[list(range(NC))],
                            ins=[(a2a_in0 if fh == 0 else a2a_in1).opt()],
                            outs=[(a2a_out0 if fh == 0 else a2a_out1).opt()],
                        )
                    pend[0] = None

                def emit_attn(b, h):
                    # score chunks processed in PAIRS: two sc matmuls land in
                    # the two adjacent PSUM banks of one [P,2,TB] tile, ONE
                    # exp activation covers both (halves the scalar-engine
                    # per-op overhead that paces the long chains), and the
                    # denominator partial sums accumulate pairwise on DVE.
                    # The av matmuls for pair p-1 are emitted after the sc
                    # pair p so exp latency stays hidden (PE in-order).
                    for qc in range(4):
                        qslc = slice(qc * TB, (qc + 1) * TB)
                        avp_ = avps.tile(
                            [P, TB], f32, tag="av", name=f"av{b}{h}{qc}"
                        )
                        nkc = 4 * qc + 4
                        npr = nkc // 2
                        exsum2 = exsp.tile(
                            [P, 2, TB], f16, tag="exs", name=f"exs{b}{h}{qc}"
                        )
                        exsum = exsp.tile(
                            [P, TB], f16, tag="exsf", name=f"exsf{b}{h}{qc}"
                        )
                        scpair = {}

                        def emit_body(p):
                            scp2 = scpair.pop(p)
                            ex2 = exps.tile(
                                [P, 2, TB], f16, tag="ex", name=f"ex{b}{h}{qc}_{p}"
                            )
                            nc.scalar.activation(
                                ex2, scp2, AF.Exp, scale=inv_sqrt_dh
                            )
                            for j in range(2):
                                kc = 2 * p + j
                                if kc >= 4 * qc:
                                    nc.vector.tensor_mul(
                                        ex2[:, j, :], ex2[:, j, :],
                                        maskt[:, kc - 4 * qc, :],
                                    )
                            for j in range(2):
                                kc = 2 * p + j
                                nc.tensor.matmul(
                                    avp_,
                                    VV[b][:, kc, h, :],
                                    ex2[:, j, :],
                                    start=(kc == 0),
                                    stop=(kc == nkc - 1),
                                )
                            if p == 0:
                                nc.vector.tensor_copy(exsum2, ex2)
                            else:
                                nc.vector.tensor_tensor(exsum2, exsum2, ex2, OP.add)

                        for p in range(npr):
                            scp2 = scps.tile(
                                [P, 2, TB], f32, tag="sc", name=f"sc{b}{h}{qc}_{p}"
                            )
                            scpair[p] = scp2
                            for j in range(2):
                                kc = 2 * p + j
                                nc.tensor.matmul(
                                    scp2[:, j, :],
                                    KK[b, h][:, kc * P : (kc + 1) * P],
                                    QR[b, h][:, qslc],
                                    start=True,
                                    stop=True,
                                )
                            if p >= 1:
                                emit_body(p - 1)
                            if p == 1:
                                flush_tail()
                            elif p == 2:
                                flush_tail1b()
                            elif p == 3:
                                flush_tail2()
                        emit_body(npr - 1)
                        nc.vector.tensor_tensor(
                            exsum, exsum2[:, 0, :], exsum2[:, 1, :], OP.add
                        )
                        flush_tail1b()
                        flush_tail2()
                        pend[0] = ("s1", b, h, qc, exsum, avp_)

                def finish():
                    flush_tail()
                    flush_tail1b()
                    flush_tail2()

                return emit_attn, finish

            # emission order: startup-critical DMAs (x rows for stats, then
            # wq/rope tables, then the first xT block) go first; b1 stats
            # pipeline under b0's projections so nothing stalls at the batch
            # boundary
            for b in range(B):
                for h in range(HPC):
                    QR[b, h] = qrk.tile(
                        [P, S], f16, tag=f"qr{b}{h}", name=f"qr{b}_{h}"
                    )
                    KK[b, h] = qrk.tile(
                        [P, S], f16, tag=f"kk{b}{h}", name=f"kk{b}_{h}"
                    )
                VV[b] = vsb.tile(
                    [P, KC, HPC, P], f16, tag=f"v{b}", name=f"vv{b}"
                )
            emit_stats(0, 0)
            nc.sync.dma_start(wq_sb, wq.rearrange("(c p) m -> p c m", p=P))
            nc.sync.dma_start(cosq, cos_qt.ap())
            nc.sync.dma_start(sinq, sin_qt.ap())
            xT00 = load_xT(0, 0)
            nc.sync.dma_start(wk_sb, wk.rearrange("(c p) m -> p c m", p=P))
            emit_stats(0, 1)
            xT01 = load_xT(0, 1)
            nc.sync.dma_start(wv_sb, wv.rearrange("(c p) m -> p c m", p=P))
            nc.sync.dma_start(cosv, cos_v.rearrange("(i p) f -> p i f", p=P))
            nc.sync.dma_start(sinv, sin_v.rearrange("(i p) f -> p i f", p=P))
            nc.sync.dma_start(maskt, masks.rearrange("m p t -> p m t"))
            emit_proj(0, 0, xT00)
            nc.sync.dma_start(b1_sb, b1s.ap())
            emit_stats(0, 2)
            xT02 = load_xT(0, 2)
            emit_proj(0, 1, xT01)
            emit_stats(0, 3)
            xT03 = load_xT(0, 3)
            emit_proj(0, 2, xT02)
            emit_stats(1, 0)
            emit_proj(0, 3, xT03)
            for qc in range(4):
                if qc < 3:
                    emit_stats(1, qc + 1)
                emit_proj(1, qc)
            # W_O pass-A weights: no dependencies, land during attention at
            # full HBM bandwidth (before the collectives run)
            nc.sync.dma_start(woeAt, wo.rearrange("(c p) e -> p c e", p=P)[:, 0:8, :])
            pj.close()

            # ---------- attention, head-major: h0 for both batches first so
            # the h=0 AllToAll fires at ~50% of attention and hides under the
            # h=1 chains; only the h=1 AllToAll tail is exposed (covered by
            # W_O pass A) ----------
            with ExitStack() as pa:
                exps = pa.enter_context(tc.tile_pool(name="exps", bufs=6))
                exsp = pa.enter_context(tc.tile_pool(name="exsp", bufs=2))
                rdBp = pa.enter_context(tc.tile_pool(name="rdBp", bufs=2))
                stg = pa.enter_context(tc.tile_pool(name="stg", bufs=3))
                scps = pa.enter_context(
                    tc.tile_pool(name="scps", bufs=2, space="PSUM")
                )
                avps = pa.enter_context(
                    tc.tile_pool(name="avps", bufs=3, space="PSUM")
                )
                dnps = pa.enter_context(
                    tc.tile_pool(name="dnps", bufs=1, space="PSUM")
                )
                emit_attn, attn_finish = make_attn(
                    exps, exsp, rdBp, stg, scps, avps, dnps
                )
                emit_attn(0, 0)
                emit_attn(1, 0)
                emit_attn(0, 1)
                emit_attn(1, 1)
                attn_finish()
                # token copy: makes the hoTe DMA region-dependent on the LAST
                # attention staging tile, so the scheduler cannot float its
                # doorbell (which waits on the h0 AllToAll) ahead of the
                # remaining attention work on the scalar queue
                nc.vector.tensor_copy(
                    hoTe[0:1, 0, 0:2], (tok_st[0] or last_st[0])[0:1, 0:2]
                )

        # ---------- phases 3+4 (transposed): W_O + residual + rmsnorm2 + MLP
        h2Tt = h2Tp.tile([P, KC, TB], f16)
        with ExitStack() as p34:
            x2p = p34.enter_context(tc.tile_pool(name="x2p", bufs=1))
            scr2 = p34.enter_context(tc.tile_pool(name="scr2", bufs=3))
            sm2 = p34.enter_context(tc.tile_pool(name="sm2", bufs=1))
            b2p = p34.enter_context(tc.tile_pool(name="b2p", bufs=1))
            outp = p34.enter_context(tc.tile_pool(name="outp", bufs=2))
            wops = p34.enter_context(tc.tile_pool(name="wops", bufs=3, space="PSUM"))
            ssps = p34.enter_context(tc.tile_pool(name="ssps", bufs=1, space="PSUM"))

            x2Tt = x2p.tile([P, KC, TB], f16)
            b2T = b2p.tile([P, KC], f32)
            nc.sync.dma_start(b2T, b2s.ap())
            wov = wo.rearrange("(c p) e -> p c e", p=P)
            with ExitStack() as p3:
                hoTp = p3.enter_context(tc.tile_pool(name="hoT", bufs=1))
                xrTp = p3.enter_context(tc.tile_pool(name="xrT", bufs=1))
                woep = p3.enter_context(tc.tile_pool(name="woe", bufs=1))

                # hoT staging rides the (idle) scalar engine's DMA queue so
                # its wait on the collectives can't block the a2a staging or
                # weight-prefetch DMAs on the sync queue
                nc.scalar.dma_start(hoTe, a2a_out0.rearrange("n p t -> p n t"))
                hoTo = hoTp.tile([P, 8, TB], f16)
                nc.scalar.dma_start(hoTo, a2a_out1.rearrange("n p t -> p n t"))
                xrT = xrTp.tile([P, KC, TB], f16)
                nc.sync.dma_start(xrT, xrT_d.rearrange("(c p) t -> p c t", p=P))
                woeBt = woep.tile([P, 8, D], f16)
                nc.sync.dma_start(woeBt, wov[:, 8:16, :])
                # pass A: even-head d-chunks (weights + chunks preloaded);
                # runs under the h=1 AllToAll tail
                for e in range(KC):
                    wp = wops.tile([P, TB], f32, tag="wo", name=f"woA{e}")
                    for j in range(8):
                        nc.tensor.matmul(
                            wp,
                            woeAt[:, j, e * P : (e + 1) * P],
                            hoTe[:, j, :],
                            start=(j == 0),
                            stop=(j == 7),
                        )
                    nc.vector.tensor_tensor(
                        x2Tt[:, e, :], wp, xrT[:, e, :], OP.add
                    )
                # pass B: odd-head d-chunks (h=1 AllToAll); the rmsnorm2
                # square/column-reduce chain trails one e-chunk behind so the
                # ones-matmul never stalls the in-order PE queue
                ssq2 = ssps.tile([P, TB], f32)

                def emit_sq(e):
                    s2 = scr2.tile([P, TB], f16, tag="s2", name=f"s2_{e}")
                    nc.scalar.activation(s2, x2Tt[:, e, :], AF.Square)
                    nc.tensor.matmul(
                        ssq2, ones_sq, s2, start=(e == 0), stop=(e == KC - 1)
                    )

                for e in range(KC):
                    wp = wops.tile([P, TB], f32, tag="wo", name=f"woB{e}")
                    for j in range(8):
                        nc.tensor.matmul(
                            wp,
                            woeBt[:, j, e * P : (e + 1) * P],
                            hoTo[:, j, :],
                            start=(j == 0),
                            stop=(j == 7),
                        )
                    nc.vector.tensor_tensor(
                        x2Tt[:, e, :], x2Tt[:, e, :], wp, OP.add
                    )
                    if e >= 1:
                        emit_sq(e - 1)
                emit_sq(KC - 1)
                rms2 = sm2.tile([P, TB], f32, tag="rms2")
                nc.scalar.activation(
                    rms2, ssq2, AF.Sqrt, bias=eps_t, scale=float(1.0 / D)
                )
                rdB2 = sm2.tile([P, TB], f32, tag="rdB2")
                nc.vector.reciprocal_approx_fast(rdB2, rms2)
                rdB2b = rdB2[:, None, :].to_broadcast([P, 8, TB])
                nc.vector.tensor_mul(h2Tt[:, 0:8, :], x2Tt[:, 0:8, :], rdB2b)
                for e in range(8, KC):
                    nc.gpsimd.tensor_mul(
                        h2Tt[:, e, :], x2Tt[:, e, :], rdB2
                    )
                # fold B2 into x2 AFTER h2 is derived (out = x2 + B2 + mlp)
                for e in range(KC):
                    nc.vector.tensor_scalar_add(
                        x2Tt[:, e, :], x2Tt[:, e, :], b2T[:, e : e + 1]
                    )

            # ---------- MLP1 ----------
            # gtt opens after phase 3 closes so it can reuse that SBUF region
            gtp = p34.enter_context(tc.tile_pool(name="gtp", bufs=1))
            gtt = gtp.tile([P, FC, TB], f16)
            # w2p opens before w1p so its region is fresh: the first MLP2
            # weight DMAs don't carry a WAR dependency on the last w1 reads
            w2p = p34.enter_context(tc.tile_pool(name="w2p", bufs=3))
            m1ctx = ExitStack()
            w1p = m1ctx.enter_context(tc.tile_pool(name="w1p", bufs=2))
            m1ps = m1ctx.enter_context(tc.tile_pool(name="m1ps", bufs=3, space="PSUM"))
            for fg in range(16):
                w1t = w1p.tile([P, KC, TB], f16, tag="w1", name=f"w1_{fg}")
                nc.sync.dma_start(w1t, w1r.ap()[fg])
                for fs in range(4):
                    f = fg * 4 + fs
                    mp = m1ps.tile([P, TB], f32, tag="m1", name=f"m1_{f}")
                    for d in range(KC):
                        nc.tensor.matmul(
                            mp,
                            w1t[:, d, fs * P : (fs + 1) * P],
                            h2Tt[:, d, :],
                            start=(d == 0),
                            stop=(d == KC - 1),
                        )
                    nc.scalar.activation(
                        gtt[:, f, :], mp, AF.Relu, bias=b1_sb[:, f : f + 1]
                    )
            m1ctx.close()

            # ---------- MLP2 (transposed output) ----------
            m2ctx = ExitStack()
            m2ps = m2ctx.enter_context(tc.tile_pool(name="m2ps", bufs=2, space="PSUM"))
            outv = out_d.rearrange("(c p) t -> p c t", p=P)
            for e in range(KC):
                w2a = w2p.tile([P, 32, P], f16, tag="w2", name=f"w2a_{e}")
                nc.sync.dma_start(
                    w2a, w2r.ap()[e][:, 0:32, :]
                )
                w2b = w2p.tile([P, 32, P], f16, tag="w2", name=f"w2b_{e}")
                nc.sync.dma_start(
                    w2b, w2r.ap()[e][:, 32:64, :]
                )
                mT = m2ps.tile([P, TB], f32, tag="m2", name=f"m2_{e}")
                for f in range(FC):
                    w2t = w2a if f < 32 else w2b
                    nc.tensor.matmul(
                        mT,
                        w2t[:, f % 32, :],
                        gtt[:, f, :],
                        start=(f == 0),
                        stop=(f == FC - 1),
                    )
                outt = outp.tile([P, TB], f32, tag="out", name=f"out{e}")
                nc.vector.tensor_tensor(outt, mT, x2Tt[:, e, :], OP.add)
                nc.sync.dma_start(outv[:, e, :], outt)
            m2ctx.close()

    nc.compile()
    return nc


def _host_inputs(inputs):
    x = np.asarray(inputs["x"], np.float32)
    Wq = np.asarray(inputs["Wq"], np.float32)
    Wk = np.asarray(inputs["Wk"], np.float32)
    Wv = np.asarray(inputs["Wv"], np.float32)
    W_O = np.asarray(inputs["W_O"], np.float32)
    scale1 = np.asarray(inputs["scale1"], np.float32)
    scale2 = np.asarray(inputs["scale2"], np.float32)
    W1 = np.asarray(inputs["W1"], np.float32)
    B1 = np.asarray(inputs["B1"], np.float32)
    W2 = np.asarray(inputs["W2"], np.float32)
    B2 = np.asarray(inputs["B2"], np.float32)

    perm = np.concatenate([np.arange(0, DH, 2), np.arange(1, DH, 2)])
    # fold rmsnorm scales into the following matmuls
    Wq_s = Wq * scale1[None, :, None]
    Wk_s = Wk * scale1[None, :, None]
    Wv_s = Wv * scale1[None, :, None]
    W1_s = W1 * scale2[:, None]

    # W_O rows reordered to match the permuted, head-major layout of HO.T,
    # with the h=0 (even) heads' row blocks first, then the h=1 (odd) heads
    # (matching the a2a_out0 / a2a_out1 arrival order)
    head_order = list(range(0, H, 2)) + list(range(1, H, 2))
    row_order = np.concatenate([h * DH + perm for h in head_order])
    wo_c = np.ascontiguousarray(W_O[row_order, :]).astype(np.float16)
    w1_c = W1_s.astype(np.float16)
    w2_c = W2.astype(np.float16)
    b1s_c = np.ascontiguousarray(B1.reshape(FC, P).T)
    b2s_c = np.ascontiguousarray(B2.reshape(KC, P).T)

    xflat = x.reshape(B * S, D)
    xf_c = xflat.astype(np.float16)
    xT_c = np.ascontiguousarray(xflat.T).astype(np.float16)
    # block-sliced xT slabs: full-length DMA lines (16KB vs 1KB)
    xT3_c = np.ascontiguousarray(
        xT_c.reshape(KC, P, B * S)
        .reshape(KC, P, B * 4, TB)
        .transpose(2, 1, 0, 3)
    )
    w1r_c = np.ascontiguousarray(
        w1_c.reshape(KC, P, 16, TB).transpose(2, 1, 0, 3)
    )
    w2r_c = np.ascontiguousarray(
        w2_c.reshape(FC, P, KC, P).transpose(2, 1, 0, 3)
    )

    # rope tables
    pos = np.arange(S, dtype=np.float64)
    pidx = np.arange(64, dtype=np.float64)
    theta_p = 1.0 / THETA ** (2.0 * pidx / DH)
    ang = pos[None, :] * theta_p[:, None]  # [64, S]
    cos_qt_c = np.cos(ang).astype(np.float16)
    sin_qt_c = np.sin(ang).astype(np.float16)
    cos_v_c = np.ascontiguousarray(cos_qt_c.T)
    sin_v_c = np.ascontiguousarray(sin_qt_c.T)

    ii = np.arange(P)[:, None]
    jj = np.arange(TB)[None, :]
    masks_c = np.stack(
        [(ii + P * m <= jj).astype(np.float16) for m in range(4)]
    )

    in_maps = []
    for c in range(NC):
        b, r = c // 4, c % 4
        heads = [HPC * c, HPC * c + 1]
        wq_c = np.concatenate([Wq_s[h][:, perm] for h in heads], 1).astype(np.float16)
        wk_c = np.concatenate([Wk_s[h][:, perm] for h in heads], 1).astype(np.float16)
        wv_c = np.concatenate([Wv_s[h][:, perm] for h in heads], 1).astype(np.float16)
        tok0 = b * S + r * TB
        in_maps.append(
            {
                "xf": xf_c,
                "xrT": np.ascontiguousarray(xT_c[:, tok0 : tok0 + TB]),
                "wq": np.ascontiguousarray(wq_c),
                "wk": np.ascontiguousarray(wk_c),
                "wv": np.ascontiguousarray(wv_c),
                "wo": wo_c,
                "xT3": xT3_c,
                "w1r": w1r_c,
                "w2r": w2r_c,
                "b1s": b1s_c,
                "b2s": b2s_c,
                "cos_qt": cos_qt_c,
                "sin_qt": sin_qt_c,
                "cos_v": cos_v_c,
                "sin_v": sin_v_c,
                "masks": masks_c,
            }
        )
    return in_maps


def kernel(**inputs):
    from concourse.bass_utils import run_bass_kernel_spmd

    trace = bool(os.environ.get("BASS_KERNEL_TRACE"))
    if trace:
        _install_ntff_hook()

    if "nc" not in _CACHE:
        _CACHE["nc"] = _build()
    nc = _CACHE["nc"]

    in_maps = _host_inputs(inputs)
    r = run_bass_kernel_spmd(nc, in_maps, list(range(NC)), trace=trace)
    kernel.last_exec_time_ns = r.exec_time_ns

    out = np.empty((B, S, D), np.float32)
    for c in range(NC):
        b, rr = c // 4, c % 4
        out[b, rr * TB : (rr + 1) * TB, :] = r.results[c]["out"].T
    return out


kernel.last_exec_time_ns = None
